# revision 5
# baseline (speedup 1.0000x reference)
"""EngagementBiasedMHA on 8 Trainium2 NeuronCores.

Sharding: 4 batches x 2 head-groups (8 heads each).  Each core computes, for
its (batch, head-group):
  - K^T projection in [feat, token] layout and V projection in [token, feat]
    layout; V is stored per key-tile as [ones(64) | V_h] so the PV
    matmul also produces the softmax denominator on partitions 0:64
  - per 512-query chunk: attention in transposed layout: S^T = K @ Q^T with
    keys on partitions, so the per-key engagement bias/mask folds into the
    Exp activation as a per-partition bias, and exp(S^T) is already the
    correct (lhs-contraction) layout for the PV matmul
  - O^T = Vhat^T @ P^T accumulated over key tiles (rows 0:64 = replicated
    softmax denominator, rows 64:128 = head output)
  - row-parallel partial output projection y_partial = O_hg @ out_w.T[hg]
Matmul operands are bf16 (4x PE throughput vs fp32); accumulation stays fp32.

Schedule: the kernel is ACT(exp)-bound in steady state (256 exps of
[128,1024] at ~1.1us each).  All projection work except a minimal prologue
(K feats m=4 chunk 0 + Q feats m=0 chunk 0) is folded into the attention
loop as deadline-placed filler matmuls so the exp stream starts ~6us in and
the PE backfills projections in the per-period slack.  K/Q bias-adds run on
DVE (not ACT) so ACT does exps only.  Next-block S^T pairs are emitted
before evac/boundary work to avoid hp-boundary exp bubbles.

Host side: transpose/slice inputs per core, then sum the two partial outputs
per batch (row-parallel unshard).
"""

import sys

if "/opt/trn_rl_repo" not in sys.path:
    sys.path.insert(0, "/opt/trn_rl_repo")

import numpy as np
from concourse import bacc, tile
import concourse.mybir as mybir
from concourse.bass_utils import run_bass_kernel_spmd

F32 = mybir.dt.float32
BF16 = mybir.dt.bfloat16
NP_BF16 = mybir.dt.np(BF16)
AF = mybir.ActivationFunctionType

B, T, D, H = 4, 2048, 1024, 16
HD = 64
HG = 8           # heads per core
NKT = T // 128   # 16 key/token tiles
NQC = T // 512   # 4 query chunks
NDT = D // 128   # 8 d_in tiles
VROW = HG * 128  # 1024 Vhat columns per key tile: per head [ones(64) | V(64)]

_cache = {}

# Results of the most recent run (for the test harness to read exec times).
last_results = None


def _build_program():
    nc = bacc.Bacc("TRN2", target_bir_lowering=False, debug=False, num_devices=8)
    xt_d = nc.declare_dram_parameter("xt", [D, T], BF16, isOutput=False)
    # wqk: row block m*128+p holds, at col d*128+f, weight qkv_w.T[d*128+p, feat(m)+f]
    wqk_d = nc.declare_dram_parameter("wqk", [1024, 1024], BF16, isOutput=False)
    wv_d = nc.declare_dram_parameter("wv", [D, 512], BF16, isOutput=False)
    bqk_d = nc.declare_dram_parameter("bqk", [128, 8], F32, isOutput=False)
    bv_d = nc.declare_dram_parameter("bv", [128, 512], F32, isOutput=False)
    eng_d = nc.declare_dram_parameter("eng", [128, NKT], F32, isOutput=False)
    maskf_d = nc.declare_dram_parameter("maskf", [128, NKT], F32, isOutput=False)
    wo_d = nc.declare_dram_parameter("wo", [512, 1024], BF16, isOutput=False)
    bo_d = nc.declare_dram_parameter("bo", [128, 1024], F32, isOutput=False)
    y_d = nc.declare_dram_parameter("y", [T, D], F32, isOutput=True)

    with tile.TileContext(nc) as tc:
        with (
            tc.tile_pool(name="persist", bufs=1) as persist,
            tc.tile_pool(name="wvpool", bufs=1) as wvpool,
            tc.tile_pool(name="wopool", bufs=1) as wopool,
            tc.tile_pool(name="small", bufs=1) as small,
            tc.tile_pool(name="ptpool", bufs=10) as ptpool,
            tc.tile_pool(name="otpool", bufs=9) as otpool,
            tc.tile_pool(name="evacpool", bufs=3) as evacpool,
            tc.tile_pool(name="recpool", bufs=3) as recpool,
            tc.tile_pool(name="psmix", bufs=2, space="PSUM") as psmix,
            tc.tile_pool(name="psops", bufs=2, space="PSUM") as psops,
            tc.tile_pool(name="psST", bufs=2, space="PSUM") as psST,
        ):
            # ---- small inputs ----
            BQK = small.tile([128, 8], F32, name="BQK")
            nc.sync.dma_start(BQK[:], bqk_d[:])
            BV = small.tile([128, 512], F32, name="BV")
            nc.sync.dma_start(BV[:], bv_d[:])
            ENG = small.tile([128, NKT], F32, name="ENG")
            nc.sync.dma_start(ENG[:], eng_d[:])
            MSK = small.tile([128, NKT], F32, name="MSK")
            nc.sync.dma_start(MSK[:], maskf_d[:])
            BO = small.tile([128, 1024], F32, name="BO")
            nc.sync.dma_start(BO[:], bo_d[:])

            # ---- per-key bias: BK = ln(max(eng, 1e-6)) - 1e9 * mask ----
            BK = small.tile([128, NKT], F32, name="BK")
            nc.vector.tensor_scalar_max(BK[:], ENG[:], 1e-6)
            nc.scalar.activation(BK[:], BK[:], AF.Ln)
            MK9 = small.tile([128, NKT], F32, name="MK9")
            nc.vector.tensor_scalar_mul(MK9[:], MSK[:], -1e9)
            nc.vector.tensor_add(BK[:], BK[:], MK9[:])

            # ---- resident activations / weights (bf16) ----
            XT = persist.tile([128, NDT * T], BF16, name="XT")
            WQK = persist.tile([128, 8 * 1024], BF16, name="WQK")
            WV = wvpool.tile([128, NDT * 512], BF16, name="WV")
            WO = wopool.tile([128, 4 * 1024], BF16, name="WO")

            def dma_wqk(m):
                nc.sync.dma_start(WQK[:, m * 1024:(m + 1) * 1024],
                                  wqk_d[m * 128:(m + 1) * 128, :])

            def dma_xt_chunk(c):
                for d in range(NDT):
                    nc.sync.dma_start(XT[:, d * T + c * 512: d * T + (c + 1) * 512],
                                      xt_d[d * 128:(d + 1) * 128, c * 512:(c + 1) * 512])

            # DMA order chosen so the prologue (K m=4 c0, Q m=0 c0) starts
            # ASAP and each interleaved projection's weights land before its
            # emission point in the attention loop.
            dma_wqk(4)
            dma_wqk(0)
            dma_xt_chunk(0)
            dma_xt_chunk(1)
            for d in range(NDT):
                nc.sync.dma_start(WV[:, d * 512:(d + 1) * 512],
                                  wv_d[d * 128:(d + 1) * 128, :])
            dma_wqk(5)
            dma_xt_chunk(2)
            dma_wqk(1)
            dma_xt_chunk(3)
            dma_wqk(6)
            dma_wqk(7)
            dma_wqk(2)
            dma_wqk(3)
            for f in range(4):
                nc.sync.dma_start(WO[:, f * 1024:(f + 1) * 1024],
                                  wo_d[f * 128:(f + 1) * 128, :])

            QTKT = persist.tile([128, 8 * T], BF16, name="QTKT")
            VHAT = persist.tile([128, NKT * VROW], BF16, name="VHAT")
            # per-key-tile memsets so V adds unblock early
            for t in range(NKT):
                nc.gpsimd.memset(VHAT[:, t * VROW:(t + 1) * VROW], 1.0)

            # ---- projection helpers (bias-add on DVE, not ACT) ----
            def proj_qk(m, c):
                # K feats (m=4..7) or Q feats (m=0..3) for token chunk c
                ps = psmix.tile([128, 512], F32, name="ps_qk", tag="mix")
                for d in range(NDT):
                    nc.tensor.matmul(
                        ps[:],
                        lhsT=WQK[:, m * 1024 + d * 128: m * 1024 + (d + 1) * 128],
                        rhs=XT[:, d * T + c * 512: d * T + c * 512 + 512],
                        start=(d == 0), stop=(d == NDT - 1),
                    )
                nc.vector.tensor_scalar_add(
                    QTKT[:, m * T + c * 512: m * T + c * 512 + 512],
                    ps[:], BQK[:, m:m + 1])

            def proj_v(t, pair):
                # V feats for heads [4*pair, 4*pair+4), token tile t (N=256)
                ps = psmix.tile([128, 256], F32, name="ps_v", tag="mix")
                for d in range(NDT):
                    nc.tensor.matmul(
                        ps[:],
                        lhsT=XT[:, d * T + t * 128: d * T + (t + 1) * 128],
                        rhs=WV[:, d * 512 + pair * 256: d * 512 + (pair + 1) * 256],
                        start=(d == 0), stop=(d == NDT - 1),
                    )
                vslice = VHAT[:, t * VROW + pair * 512: t * VROW + (pair + 1) * 512
                              ].rearrange("p (h c) -> p h c", c=128)[:, :, 64:128]
                nc.vector.tensor_add(
                    vslice,
                    ps[:].rearrange("p (h c) -> p h c", c=64),
                    BV[:, pair * 256:(pair + 1) * 256].rearrange(
                        "p (h c) -> p h c", c=64))

            def out_proj(qc2, otc2, grp):
                t4, c2 = grp // 2, grp % 2
                tt = qc2 * 4 + t4
                ps = psmix.tile([128, 512], F32, name="ps_y", tag="mix")
                for f in range(4):
                    nc.tensor.matmul(
                        ps[:],
                        lhsT=otc2[f][:, t4 * 128:(t4 + 1) * 128],
                        rhs=WO[:, f * 1024 + c2 * 512: f * 1024 + c2 * 512 + 512],
                        start=(f == 0), stop=(f == 3))
                yv = evacpool.tile([128, 512], F32, name="yv", tag="yv")
                nc.vector.tensor_add(yv[:], ps[:], BO[:, c2 * 512:(c2 + 1) * 512])
                nc.sync.dma_start(
                    y_d[tt * 128:(tt + 1) * 128, c2 * 512:(c2 + 1) * 512], yv[:])

            # ---- prologue: just enough for the exp stream to start ----
            proj_qk(4, 0)   # K feats for hp0, token chunk 0
            proj_qk(0, 0)   # Q feats for qt0, query chunk 0

            # ---- filler schedule: (qc, hp, kt) -> list of thunks ----
            fillers = {}

            def add_filler(qc, hp, kt, thunk):
                fillers.setdefault((qc, hp, kt), []).append(thunk)

            # K m=4 remaining chunks inside (qc0, hp0); needed by S(hp0, kt>=4c)
            for c in range(1, 4):
                add_filler(0, 0, 4 * c - 2, (lambda c=c: proj_qk(4, c)))
            # K m=5/6/7 chunk-granular inside hp0/1/2 of qc0
            for j, m in enumerate((5, 6, 7)):
                for c in range(4):
                    add_filler(0, j, 4 * c + 1, (lambda m=m, c=c: proj_qk(m, c)))
            # Q m=1/2/3 for qc0 at the tail of hp0/1/2
            for j, m in enumerate((1, 2, 3)):
                add_filler(0, j, 14, (lambda m=m: proj_qk(m, 0)))
            # V projection: pair0 (heads 0..3) during hp0, pair1 during hp2
            for t in range(NKT):
                add_filler(0, 0, t, (lambda t=t: proj_v(t, 0)))
                add_filler(0, 2, t, (lambda t=t: proj_v(t, 1)))

            # ---- attention: flattened (qc, hp, kt) pipeline ----
            # Per index i: S-pair, exp, fillers, PV-pair.  Block-boundary
            # work (evac of ops, deferred out_proj, q_proj of next qc) is
            # emitted at kt==1 of the FOLLOWING block so the next S-pair is
            # already in flight when the boundary work runs.
            blocks = [(qc, hp) for qc in range(NQC) for hp in range(4)]
            state = {}   # per-block live tiles: ops, pt list
            otc_by_qc = {}
            prev_out = {"qc": None, "otc": None, "grp": 0}

            def emit_block_tail(bi):
                qc, hp = blocks[bi]
                ops = state.pop((qc, hp))["ops"]
                OTc = otpool.tile([128, 512], BF16, name="OTc", tag="otc")
                for sub in range(2):
                    rec = recpool.tile([64, 512], F32, name="rec", tag="rec")
                    nc.vector.reciprocal_approx_fast(rec[:], ops[sub][0:64, :])
                    nc.vector.tensor_mul(
                        OTc[sub * 64:sub * 64 + 64, :],
                        ops[sub][64:128, :], rec[:])
                otc_by_qc.setdefault(qc, []).append(OTc)
                # deferred out-proj of the previous query chunk (2 groups)
                if prev_out["qc"] is not None and prev_out["grp"] < 8:
                    g = prev_out["grp"]
                    out_proj(prev_out["qc"], prev_out["otc"], g)
                    out_proj(prev_out["qc"], prev_out["otc"], g + 1)
                    prev_out["grp"] = g + 2
                # q-proj of the next query chunk
                if qc + 1 < NQC:
                    proj_qk(hp, qc + 1)
                if hp == 3:
                    # roll the deferred-out-proj window to this qc
                    prev_out["qc"] = qc
                    prev_out["otc"] = otc_by_qc[qc]
                    prev_out["grp"] = 0

            for i in range(len(blocks) * NKT):
                bi, kt = i // NKT, i % NKT
                qc, hp = blocks[bi]
                qt = hp
                ktf = 4 + hp
                if kt == 0:
                    op0 = psops.tile([128, 512], F32, name="op0", tag="ops")
                    op1 = psops.tile([128, 512], F32, name="op1", tag="ops")
                    state[(qc, hp)] = {"ops": (op0, op1)}
                ops = state[(qc, hp)]["ops"]

                # S^T pair (the two K=64 matmuls run concurrently via
                # base_partition-derived PE row tiling)
                st = psST.tile([128, 1024], F32, name="st", tag="st")
                for sub in range(2):
                    lo = sub * 64
                    nc.tensor.matmul(
                        st[:, sub * 512:(sub + 1) * 512],
                        lhsT=QTKT[lo:lo + 64, ktf * T + kt * 128: ktf * T + (kt + 1) * 128],
                        rhs=QTKT[lo:lo + 64, qt * T + qc * 512: qt * T + qc * 512 + 512],
                        start=True, stop=True)
                pt = ptpool.tile([128, 1024], BF16, name="pt", tag="pt")
                nc.scalar.activation(
                    pt[:], st[:], AF.Exp,
                    bias=BK[:, kt:kt + 1], scale=0.125)

                # boundary work of the previous block, after this block's
                # first S-pair is already emitted
                if kt == 1 and bi > 0:
                    emit_block_tail(bi - 1)

                for th in fillers.get((qc, hp, kt), ()):
                    th()

                for sub in range(2):
                    h = 2 * hp + sub
                    nc.tensor.matmul(
                        ops[sub][:],
                        lhsT=VHAT[:, kt * VROW + h * 128: kt * VROW + (h + 1) * 128],
                        rhs=pt[:, sub * 512:(sub + 1) * 512],
                        start=(kt == 0), stop=(kt == NKT - 1))

            emit_block_tail(len(blocks) - 1)
            # all of qc3's out-projs drain at the end
            for grp in range(8):
                out_proj(3, otc_by_qc[3], grp)
    nc.compile()
    return nc


def get_program():
    if "nc" not in _cache:
        _cache["nc"] = _build_program()
    return _cache["nc"]


def shard_inputs(x, engagement, mask, qkv_w, qkv_b, out_w, out_b):
    """Build the per-core input maps (host-side layout prep only)."""
    x = np.asarray(x, dtype=np.float32)
    engagement = np.asarray(engagement, dtype=np.float32)
    maskf = np.asarray(mask).astype(np.float32)
    qkv_w = np.asarray(qkv_w, dtype=np.float32)
    qkv_b = np.asarray(qkv_b, dtype=np.float32)
    out_w = np.asarray(out_w, dtype=np.float32)
    out_b = np.asarray(out_b, dtype=np.float32)

    qkvT = qkv_w.T  # [D, 3D]
    outT = out_w.T  # [D, D]
    in_maps = []
    for cix in range(8):
        b, hg = cix // 2, cix % 2
        qcols = qkvT[:, hg * 512:(hg + 1) * 512]
        kcols = qkvT[:, 1024 + hg * 512: 1024 + (hg + 1) * 512]
        sel = np.concatenate([qcols, kcols], axis=1)  # [1024 din, 1024 feats]
        # [d, p, m, f] -> [m, p, d, f] -> [(m p), (d f)]
        wqk = sel.reshape(NDT, 128, 8, 128).transpose(2, 1, 0, 3).reshape(1024, 1024)
        bq = qkv_b[hg * 512:(hg + 1) * 512].reshape(4, 128).T
        bk = qkv_b[1024 + hg * 512: 1024 + (hg + 1) * 512].reshape(4, 128).T
        bo = np.broadcast_to(out_b, (128, 1024)) if hg == 0 else np.zeros((128, 1024), np.float32)
        in_maps.append({
            "xt": np.ascontiguousarray(x[b].T).astype(NP_BF16),
            "wqk": np.ascontiguousarray(wqk).astype(NP_BF16),
            "wv": np.ascontiguousarray(
                qkvT[:, 2048 + hg * 512: 2048 + (hg + 1) * 512]).astype(NP_BF16),
            "bqk": np.ascontiguousarray(np.concatenate([bq, bk], axis=1)),
            "bv": np.ascontiguousarray(
                np.broadcast_to(qkv_b[2048 + hg * 512: 2048 + (hg + 1) * 512], (128, 512))),
            "eng": np.ascontiguousarray(engagement[b].reshape(NKT, 128).T),
            "maskf": np.ascontiguousarray(maskf[b].reshape(NKT, 128).T),
            "wo": np.ascontiguousarray(outT[hg * 512:(hg + 1) * 512, :]).astype(NP_BF16),
            "bo": np.ascontiguousarray(bo),
        })
    return in_maps


def kernel(x, engagement, mask, qkv_w, qkv_b, out_w, out_b):
    global last_results
    nc = get_program()
    in_maps = shard_inputs(x, engagement, mask, qkv_w, qkv_b, out_w, out_b)
    res = run_bass_kernel_spmd(nc, in_maps, list(range(8)))
    last_results = res
    out = np.empty((B, T, D), dtype=np.float32)
    for b in range(B):
        out[b] = res.results[2 * b]["y"] + res.results[2 * b + 1]["y"]
    return out


# revision 10
# speedup vs baseline: 1.0248x; 1.0248x over previous
"""EngagementBiasedMHA on 8 Trainium2 NeuronCores.

Sharding: 4 batches x 2 head-groups (8 heads each).  Each core computes, for
its (batch, head-group):
  - K^T projection in [feat, token] layout and V projection in [token, feat]
    layout; V is stored per key-tile as [ones(64) | V_h] so the PV
    matmul also produces the softmax denominator on partitions 0:64
  - per 512-query chunk: attention in transposed layout: S^T = K @ Q^T with
    keys on partitions, so the per-key engagement bias/mask folds into the
    Exp activation as a per-partition bias, and exp(S^T) is already the
    correct (lhs-contraction) layout for the PV matmul
  - O^T = Vhat^T @ P^T accumulated over key tiles (rows 0:64 = replicated
    softmax denominator, rows 64:128 = head output)
  - row-parallel partial output projection y_partial = O_hg @ out_w.T[hg]
Matmul operands are bf16 (4x PE throughput vs fp32); accumulation stays fp32.

Schedule: the kernel is ACT(exp)-bound in steady state (256 exps of
[128,1024] at ~1.1us each).  All projection work except a minimal prologue
(K feats m=4 chunk 0 + Q feats m=0 chunk 0) is folded into the attention
loop as deadline-placed filler matmuls so the exp stream starts ~6us in and
the PE backfills projections in the per-period slack.  K/Q bias-adds run on
DVE (not ACT) so ACT does exps only.  Next-block S^T pairs are emitted
before evac/boundary work to avoid hp-boundary exp bubbles.

Host side: transpose/slice inputs per core, then sum the two partial outputs
per batch (row-parallel unshard).
"""

import sys

if "/opt/trn_rl_repo" not in sys.path:
    sys.path.insert(0, "/opt/trn_rl_repo")

import numpy as np
from concourse import bacc, tile
import concourse.mybir as mybir
from concourse.bass_utils import run_bass_kernel_spmd

F32 = mybir.dt.float32
BF16 = mybir.dt.bfloat16
NP_BF16 = mybir.dt.np(BF16)
AF = mybir.ActivationFunctionType

B, T, D, H = 4, 2048, 1024, 16
HD = 64
HG = 8           # heads per core
NKT = T // 128   # 16 key/token tiles
NQC = T // 512   # 4 query chunks
NDT = D // 128   # 8 d_in tiles
VROW = HG * 128  # 1024 Vhat columns per key tile: per head [ones(64) | V(64)]

_cache = {}

# Results of the most recent run (for the test harness to read exec times).
last_results = None


def _build_program():
    nc = bacc.Bacc("TRN2", target_bir_lowering=False, debug=False, num_devices=8)
    xt_d = nc.declare_dram_parameter("xt", [D, T], BF16, isOutput=False)
    # wqk: row block m*128+p holds, at col d*128+f, weight qkv_w.T[d*128+p, feat(m)+f]
    wqk_d = nc.declare_dram_parameter("wqk", [1024, 1024], BF16, isOutput=False)
    wv_d = nc.declare_dram_parameter("wv", [D, 512], BF16, isOutput=False)
    bqk_d = nc.declare_dram_parameter("bqk", [128, 8], F32, isOutput=False)
    bv_d = nc.declare_dram_parameter("bv", [128, 512], F32, isOutput=False)
    eng_d = nc.declare_dram_parameter("eng", [128, NKT], F32, isOutput=False)
    maskf_d = nc.declare_dram_parameter("maskf", [128, NKT], F32, isOutput=False)
    wo_d = nc.declare_dram_parameter("wo", [512, 1024], BF16, isOutput=False)
    bo_d = nc.declare_dram_parameter("bo", [128, 1024], F32, isOutput=False)
    y_d = nc.declare_dram_parameter("y", [T, D], F32, isOutput=True)

    with tile.TileContext(nc) as tc:
        with (
            tc.tile_pool(name="persist", bufs=1) as persist,
            tc.tile_pool(name="wvpool", bufs=1) as wvpool,
            tc.tile_pool(name="wopool", bufs=1) as wopool,
            tc.tile_pool(name="small", bufs=1) as small,
            tc.tile_pool(name="ptpool", bufs=12) as ptpool,
            tc.tile_pool(name="otpool", bufs=9) as otpool,
            tc.tile_pool(name="evacpool", bufs=3) as evacpool,
            tc.tile_pool(name="recpool", bufs=3) as recpool,
            tc.tile_pool(name="psmix", bufs=2, space="PSUM") as psmix,
            tc.tile_pool(name="psops", bufs=2, space="PSUM") as psops,
            tc.tile_pool(name="psST", bufs=2, space="PSUM") as psST,
        ):
            # ---- small inputs ----
            BQK = small.tile([128, 8], F32, name="BQK")
            nc.sync.dma_start(BQK[:], bqk_d[:])
            BV = small.tile([128, 512], F32, name="BV")
            nc.sync.dma_start(BV[:], bv_d[:])
            ENG = small.tile([128, NKT], F32, name="ENG")
            nc.sync.dma_start(ENG[:], eng_d[:])
            MSK = small.tile([128, NKT], F32, name="MSK")
            nc.sync.dma_start(MSK[:], maskf_d[:])
            BO = small.tile([128, 1024], F32, name="BO")
            nc.sync.dma_start(BO[:], bo_d[:])

            # ---- per-key bias: BK = ln(max(eng, 1e-6)) - 1e9 * mask ----
            BK = small.tile([128, NKT], F32, name="BK")
            nc.vector.tensor_scalar_max(BK[:], ENG[:], 1e-6)
            nc.scalar.activation(BK[:], BK[:], AF.Ln)
            MK9 = small.tile([128, NKT], F32, name="MK9")
            nc.vector.tensor_scalar_mul(MK9[:], MSK[:], -1e9)
            nc.vector.tensor_add(BK[:], BK[:], MK9[:])

            # ---- resident activations / weights (bf16) ----
            XT = persist.tile([128, NDT * T], BF16, name="XT")
            WQK = persist.tile([128, 8 * 1024], BF16, name="WQK")
            WV = wvpool.tile([128, NDT * 512], BF16, name="WV")
            WO = wopool.tile([128, 4 * 1024], BF16, name="WO")

            def dma_wqk(m, splits=1):
                # split across partition ranges -> parallel DMA queues
                step = 128 // splits
                for s in range(splits):
                    nc.sync.dma_start(
                        WQK[s * step:(s + 1) * step, m * 1024:(m + 1) * 1024],
                        wqk_d[m * 128 + s * step: m * 128 + (s + 1) * step, :])

            def dma_xt_chunk(c, splits=1):
                step = 128 // splits
                for d in range(NDT):
                    for s in range(splits):
                        nc.sync.dma_start(
                            XT[s * step:(s + 1) * step,
                               d * T + c * 512: d * T + (c + 1) * 512],
                            xt_d[d * 128 + s * step: d * 128 + (s + 1) * step,
                                 c * 512:(c + 1) * 512])

            # DMA order chosen so the prologue (K m=4 c0, Q m=0 c0) starts
            # ASAP and each interleaved projection's weights land before its
            # emission point in the attention loop.  Early critical tensors
            # are split across several queues.
            dma_wqk(4, splits=4)
            dma_wqk(0, splits=4)
            dma_xt_chunk(0, splits=2)
            dma_xt_chunk(1, splits=2)
            for d in range(NDT):
                nc.sync.dma_start(WV[:, d * 512:(d + 1) * 512],
                                  wv_d[d * 128:(d + 1) * 128, :])
            dma_wqk(5)
            dma_xt_chunk(2)
            dma_wqk(1)
            dma_xt_chunk(3)
            dma_wqk(6)
            dma_wqk(7)
            dma_wqk(2)
            dma_wqk(3)
            for f in range(4):
                nc.sync.dma_start(WO[:, f * 1024:(f + 1) * 1024],
                                  wo_d[f * 128:(f + 1) * 128, :])

            QTKT = persist.tile([128, 8 * T], BF16, name="QTKT")
            VHAT = persist.tile([128, NKT * VROW], BF16, name="VHAT")
            # per-key-tile memsets so V adds unblock early
            for t in range(NKT):
                nc.gpsimd.memset(VHAT[:, t * VROW:(t + 1) * VROW], 1.0)

            # ---- projection helpers (bias-add on DVE, not ACT) ----
            def proj_qk(m, c):
                # K feats (m=4..7) or Q feats (m=0..3) for token chunk c
                ps = psmix.tile([128, 512], F32, name="ps_qk", tag="mix")
                for d in range(NDT):
                    nc.tensor.matmul(
                        ps[:],
                        lhsT=WQK[:, m * 1024 + d * 128: m * 1024 + (d + 1) * 128],
                        rhs=XT[:, d * T + c * 512: d * T + c * 512 + 512],
                        start=(d == 0), stop=(d == NDT - 1),
                    )
                nc.vector.tensor_scalar_add(
                    QTKT[:, m * T + c * 512: m * T + c * 512 + 512],
                    ps[:], BQK[:, m:m + 1])

            def proj_v(t, pair):
                # V feats for heads [4*pair, 4*pair+4), token tile t (N=256)
                ps = psmix.tile([128, 256], F32, name="ps_v", tag="mix")
                for d in range(NDT):
                    nc.tensor.matmul(
                        ps[:],
                        lhsT=XT[:, d * T + t * 128: d * T + (t + 1) * 128],
                        rhs=WV[:, d * 512 + pair * 256: d * 512 + (pair + 1) * 256],
                        start=(d == 0), stop=(d == NDT - 1),
                    )
                vslice = VHAT[:, t * VROW + pair * 512: t * VROW + (pair + 1) * 512
                              ].rearrange("p (h c) -> p h c", c=128)[:, :, 64:128]
                nc.vector.tensor_add(
                    vslice,
                    ps[:].rearrange("p (h c) -> p h c", c=64),
                    BV[:, pair * 256:(pair + 1) * 256].rearrange(
                        "p (h c) -> p h c", c=64))

            def out_proj(qc2, otc2, grp):
                t4, c2 = grp // 2, grp % 2
                tt = qc2 * 4 + t4
                ps = psmix.tile([128, 512], F32, name="ps_y", tag="mix")
                for f in range(4):
                    nc.tensor.matmul(
                        ps[:],
                        lhsT=otc2[f][:, t4 * 128:(t4 + 1) * 128],
                        rhs=WO[:, f * 1024 + c2 * 512: f * 1024 + c2 * 512 + 512],
                        start=(f == 0), stop=(f == 3))
                yv = evacpool.tile([128, 512], F32, name="yv", tag="yv")
                nc.vector.tensor_add(yv[:], ps[:], BO[:, c2 * 512:(c2 + 1) * 512])
                nc.sync.dma_start(
                    y_d[tt * 128:(tt + 1) * 128, c2 * 512:(c2 + 1) * 512], yv[:])

            # ---- prologue: just enough for the exp stream to start ----
            proj_qk(4, 0)   # K feats for hp0, token chunk 0
            proj_qk(0, 0)   # Q feats for qt0, query chunk 0

            # ---- block order: interleave qc0/qc1 (then qc2/qc3) so the
            # projection fillers' deadlines spread over 128 periods instead
            # of crowding into the first 64 ----
            BLOCKS = [(0, 0), (0, 1), (1, 0), (1, 1), (0, 2), (0, 3), (1, 2), (1, 3),
                      (2, 0), (2, 1), (3, 0), (3, 1), (2, 2), (2, 3), (3, 2), (3, 3)]

            # ---- filler schedule: block index, kt -> list of thunks ----
            fillers = {}

            def add_filler(bi, kt, thunk):
                fillers.setdefault((bi, kt), []).append(thunk)

            # K feature tiles, chunk-granular just-in-time:
            #   m=4: prologue c0; c1/c2/c3 inside blk0 (first hp0 block)
            #   m=5: c0 at blk0 tail; c1/2/3 inside blk1 (first hp1 block)
            #   m=6: c0 at blk3 tail; c1/2/3 inside blk4 (first hp2 block)
            #   m=7: c0 at blk4 tail; c1/2/3 inside blk5 (first hp3 block)
            for c in range(1, 4):
                add_filler(0, 4 * c - 3, (lambda c=c: proj_qk(4, c)))
                add_filler(1, 4 * c - 3, (lambda c=c: proj_qk(5, c)))
                add_filler(4, 4 * c - 3, (lambda c=c: proj_qk(6, c)))
                add_filler(5, 4 * c - 3, (lambda c=c: proj_qk(7, c)))
            add_filler(0, 13, (lambda: proj_qk(5, 0)))
            add_filler(3, 13, (lambda: proj_qk(6, 0)))
            add_filler(4, 13, (lambda: proj_qk(7, 0)))
            # Q-tiles JIT: block bi needs Q(m=hp, c=qc); emit one block ahead
            for bi in range(1, 16):
                qc, hp = BLOCKS[bi]
                add_filler(bi - 1, 14, (lambda hp=hp, qc=qc: proj_qk(hp, qc)))
            # V projection: pair0 (heads 0..3) during blk0, pair1 during blk4
            for t in range(NKT):
                add_filler(0, t, (lambda t=t: proj_v(t, 0)))
                add_filler(4, t, (lambda t=t: proj_v(t, 1)))

            # deferred out-proj groups: qc ready after its last block's evac
            # (qc0 after blk5, qc1 after blk7, qc2 after blk13, qc3 at end).
            # tails run at blk+1 kt==1; qc3's 8 groups drain post-loop.
            outproj_at_tail = {
                6: [(0, (0, 1))], 7: [(0, (2, 3))], 8: [(0, (4, 5))],
                9: [(0, (6, 7))],
                10: [(1, (0, 1))], 11: [(1, (2, 3))], 12: [(1, (4, 5))],
                13: [(1, (6, 7)), (2, (0, 1))],
                14: [(2, (2, 3, 4, 5))], 15: [(2, (6, 7))],
            }

            state = {}
            otc_by_qc = {}

            def emit_block_tail(bi):
                qc, hp = BLOCKS[bi]
                ops = state.pop(bi)["ops"]
                OTc = otpool.tile([128, 512], BF16, name="OTc", tag="otc")
                for sub in range(2):
                    rec = recpool.tile([64, 512], F32, name="rec", tag="rec")
                    nc.vector.reciprocal_approx_fast(rec[:], ops[sub][0:64, :])
                    nc.vector.tensor_mul(
                        OTc[sub * 64:sub * 64 + 64, :],
                        ops[sub][64:128, :], rec[:])
                otc_by_qc.setdefault(qc, {})[hp] = OTc
                for oqc, grps in outproj_at_tail.get(bi, ()):
                    otc = [otc_by_qc[oqc][f] for f in range(4)]
                    for g in grps:
                        out_proj(oqc, otc, g)

            def pv_pair(bi, kt, pt):
                qc, hp = BLOCKS[bi]
                ops = state[bi]["ops"]
                for sub in range(2):
                    h = 2 * hp + sub
                    nc.tensor.matmul(
                        ops[sub][:],
                        lhsT=VHAT[:, kt * VROW + h * 128: kt * VROW + (h + 1) * 128],
                        rhs=pt[:, sub * 512:(sub + 1) * 512],
                        start=(kt == 0), stop=(kt == NKT - 1))

            # ---- attention: flattened pipeline over BLOCKS x kt ----
            # Per index: S-pair, exp, prev-block tail (at kt==1), fillers,
            # then the PV pair LAGGED by one iteration so it never blocks
            # the in-order PE queue waiting on the exp or the evac.
            pts = {}
            for i in range(len(BLOCKS) * NKT):
                bi, kt = i // NKT, i % NKT
                qc, hp = BLOCKS[bi]
                qt = hp
                ktf = 4 + hp
                if kt == 0:
                    op0 = psops.tile([128, 512], F32, name="op0", tag="ops")
                    op1 = psops.tile([128, 512], F32, name="op1", tag="ops")
                    state[bi] = {"ops": (op0, op1)}

                # S^T pair (the two K=64 matmuls run concurrently via
                # base_partition-derived PE row tiling)
                st = psST.tile([128, 1024], F32, name="st", tag="st")
                for sub in range(2):
                    lo = sub * 64
                    nc.tensor.matmul(
                        st[:, sub * 512:(sub + 1) * 512],
                        lhsT=QTKT[lo:lo + 64, ktf * T + kt * 128: ktf * T + (kt + 1) * 128],
                        rhs=QTKT[lo:lo + 64, qt * T + qc * 512: qt * T + qc * 512 + 512],
                        start=True, stop=True)
                pt = ptpool.tile([128, 1024], BF16, name="pt", tag="pt")
                nc.scalar.activation(
                    pt[:], st[:], AF.Exp,
                    bias=BK[:, kt:kt + 1], scale=0.125)
                pts[(bi, kt)] = pt

                if kt == 1 and bi > 0:
                    emit_block_tail(bi - 1)
                for th in fillers.get((bi, kt), ()):
                    th()

                if kt == 0:
                    if bi > 0:
                        pv_pair(bi - 1, 15, pts.pop((bi - 1, 15)))
                else:
                    pv_pair(bi, kt - 1, pts.pop((bi, kt - 1)))

            pv_pair(15, 15, pts.pop((15, 15)))
            emit_block_tail(15)
            # qc3's out-projs drain at the end
            otc3 = [otc_by_qc[3][f] for f in range(4)]
            for grp in range(8):
                out_proj(3, otc3, grp)
    nc.compile()
    return nc


def get_program():
    if "nc" not in _cache:
        _cache["nc"] = _build_program()
    return _cache["nc"]


def shard_inputs(x, engagement, mask, qkv_w, qkv_b, out_w, out_b):
    """Build the per-core input maps (host-side layout prep only)."""
    x = np.asarray(x, dtype=np.float32)
    engagement = np.asarray(engagement, dtype=np.float32)
    maskf = np.asarray(mask).astype(np.float32)
    qkv_w = np.asarray(qkv_w, dtype=np.float32)
    qkv_b = np.asarray(qkv_b, dtype=np.float32)
    out_w = np.asarray(out_w, dtype=np.float32)
    out_b = np.asarray(out_b, dtype=np.float32)

    qkvT = qkv_w.T  # [D, 3D]
    outT = out_w.T  # [D, D]
    in_maps = []
    for cix in range(8):
        b, hg = cix // 2, cix % 2
        qcols = qkvT[:, hg * 512:(hg + 1) * 512]
        kcols = qkvT[:, 1024 + hg * 512: 1024 + (hg + 1) * 512]
        sel = np.concatenate([qcols, kcols], axis=1)  # [1024 din, 1024 feats]
        # [d, p, m, f] -> [m, p, d, f] -> [(m p), (d f)]
        wqk = sel.reshape(NDT, 128, 8, 128).transpose(2, 1, 0, 3).reshape(1024, 1024)
        bq = qkv_b[hg * 512:(hg + 1) * 512].reshape(4, 128).T
        bk = qkv_b[1024 + hg * 512: 1024 + (hg + 1) * 512].reshape(4, 128).T
        bo = np.broadcast_to(out_b, (128, 1024)) if hg == 0 else np.zeros((128, 1024), np.float32)
        in_maps.append({
            "xt": np.ascontiguousarray(x[b].T).astype(NP_BF16),
            "wqk": np.ascontiguousarray(wqk).astype(NP_BF16),
            "wv": np.ascontiguousarray(
                qkvT[:, 2048 + hg * 512: 2048 + (hg + 1) * 512]).astype(NP_BF16),
            "bqk": np.ascontiguousarray(np.concatenate([bq, bk], axis=1)),
            "bv": np.ascontiguousarray(
                np.broadcast_to(qkv_b[2048 + hg * 512: 2048 + (hg + 1) * 512], (128, 512))),
            "eng": np.ascontiguousarray(engagement[b].reshape(NKT, 128).T),
            "maskf": np.ascontiguousarray(maskf[b].reshape(NKT, 128).T),
            "wo": np.ascontiguousarray(outT[hg * 512:(hg + 1) * 512, :]).astype(NP_BF16),
            "bo": np.ascontiguousarray(bo),
        })
    return in_maps


def kernel(x, engagement, mask, qkv_w, qkv_b, out_w, out_b):
    global last_results
    nc = get_program()
    in_maps = shard_inputs(x, engagement, mask, qkv_w, qkv_b, out_w, out_b)
    res = run_bass_kernel_spmd(nc, in_maps, list(range(8)))
    last_results = res
    out = np.empty((B, T, D), dtype=np.float32)
    for b in range(B):
        out[b] = res.results[2 * b]["y"] + res.results[2 * b + 1]["y"]
    return out
